# revision 11
# baseline (speedup 1.0000x reference)
"""NGCN layer on 8 trn2 NeuronCores.

Computation: support = input @ weight; 3x SpMM (out[r] = sum val[e]*h[col[e]]
over edges with row[e]==r); + bias.

Sharding: output rows (nodes) are range-partitioned across the 8 cores.  Each
core owns rows [m*12500, (m+1)*12500).  The dense matmul is row-parallel with
the [512,128] weight replicated.  Each SpMM iteration gathers arbitrary source
rows, so the full updated node matrix is AllGathered between iterations.

Per-core SpMM: edges whose dest row is owned by the core are sorted by
(dest row tile of 125, col bank of 25000, col).  For each 125-row output tile,
`dma_gather` fetches the source rows h[col[e]] (128 edges per chunk, one row
per SBUF partition; 4 gathers per tile, one per 25000-row col bank since
gather indices are int16).  A selection matrix S[e, r] = val[e]*(d[e]==r)
is built with one fused DVE tensor_scalar op per chunk from an iota row and
per-edge (d, val) metadata, and the TensorEngine accumulates
S.T @ X_gathered into PSUM across all chunks of the tile - that is the
segment-sum.  Padding slots carry val=0 / idx=0 so they contribute nothing.
"""

import math

import numpy as np

import concourse.bacc as bacc
import concourse.bass as bass
import concourse.mybir as mybir
import concourse.tile as tile
from concourse import bass_utils

F32 = mybir.dt.float32


class Cfg:
    def __init__(self, n_cores=8, n_nodes=100000, tile_rows=125, n_banks=4,
                 feat=128, kdim=512, iters=3, dma_scratch=32768,
                 use_collective=True):
        self.n_cores = n_cores
        self.n_nodes = n_nodes
        self.rows_core = n_nodes // n_cores
        self.tile_rows = tile_rows
        self.n_tiles = self.rows_core // tile_rows
        assert self.rows_core % tile_rows == 0
        self.n_banks = n_banks
        self.bank_rows = n_nodes // n_banks
        assert n_nodes % n_banks == 0
        assert self.bank_rows <= 32767, "gather idx must fit int16"
        self.feat = feat
        self.kdim = kdim
        self.iters = iters
        self.dma_scratch = dma_scratch
        self.use_collective = use_collective


DEFAULT_CFG = Cfg()


def _preprocess(cfg, input_, weight, bias, edge_val, edge_row, edge_col):
    """Build per-core input arrays. Returns (in_maps, CB)."""
    nc_, rows_core = cfg.n_cores, cfg.rows_core
    tr, nt, nb = cfg.tile_rows, cfg.n_tiles, cfg.n_banks

    per_core = []
    max_cnt = 1
    for m in range(nc_):
        lo = m * rows_core
        sel = (edge_row >= lo) & (edge_row < lo + rows_core)
        r = edge_row[sel] - lo
        c = edge_col[sel]
        v = edge_val[sel]
        t = r // tr
        d = r % tr
        b = c // cfg.bank_rows
        cb = c % cfg.bank_rows
        key = t * nb + b
        order = np.lexsort((cb, key))
        key, d, v, cb = key[order], d[order], v[order], cb[order]
        cnt = np.bincount(key, minlength=nt * nb)
        if cnt.size:
            max_cnt = max(max_cnt, int(cnt.max()))
        per_core.append((key, d, v, cb, cnt))

    CB = (max_cnt + 127) // 128  # chunks per (tile, bank)
    CB = max(CB, getattr(cfg, "min_CB", 1))
    cap = CB * 128
    S16 = cap // 16
    NCH = nb * CB

    in_maps = []
    for m in range(nc_):
        key, d, v, cb, cnt = per_core[m]
        # slot index of each edge within its (tile,bank) group
        starts = np.zeros(nt * nb, dtype=np.int64)
        np.cumsum(cnt[:-1], out=starts[1:])
        rank = np.arange(key.size, dtype=np.int64) - starts[key]
        flat = key * cap + rank

        idx_a = np.zeros(nt * nb * cap, dtype=np.int16)
        d_a = np.zeros(nt * nb * cap, dtype=np.float32)
        v_a = np.zeros(nt * nb * cap, dtype=np.float32)
        idx_a[flat] = cb.astype(np.int16)
        d_a[flat] = d.astype(np.float32)
        v_a[flat] = v.astype(np.float32)

        # idx: [nt, nb, cap] -> slot i -> (partition i%16, col i//16),
        # replicated over the 8 groups of 16 partitions.
        idx_a = idx_a.reshape(nt, nb, S16, 16).transpose(0, 1, 3, 2)  # [nt,nb,16,S16]
        idx_a = np.ascontiguousarray(
            np.broadcast_to(idx_a[:, :, None, :, :], (nt, nb, 8, 16, S16))
        ).reshape(nt, nb, 128, S16)

        # meta: [nt, 128, 2, NCH]; slot j*128+p of chunk (b*CB+j)
        d_a = d_a.reshape(nt, nb * CB, 128).transpose(0, 2, 1)  # [nt,128,NCH]
        v_a = v_a.reshape(nt, nb * CB, 128).transpose(0, 2, 1)
        meta = np.ascontiguousarray(
            np.stack([d_a, v_a], axis=2)
        ).reshape(nt, 128, 2 * NCH)

        lo = m * rows_core
        inT = np.ascontiguousarray(input_[lo:lo + rows_core].T)  # [kdim, rows]
        in_maps.append({
            "inT": inT,
            "w": np.ascontiguousarray(weight),
            "biasb": np.ascontiguousarray(
                np.broadcast_to(bias[None, :], (128, cfg.feat))),
            "iota": np.ascontiguousarray(
                np.broadcast_to(np.arange(tr, dtype=np.float32)[None, :],
                                (128, tr))),
            "gidx": np.ascontiguousarray(idx_a),
            "meta": meta,
        })
    return in_maps, CB


def _build(cfg, CB):
    tr, nt, nb = cfg.tile_rows, cfg.n_tiles, cfg.n_banks
    NCH = nb * CB
    cap = CB * 128
    S16 = cap // 16
    feat, kdim = cfg.feat, cfg.kdim
    kc_n = kdim // 128

    nc = bacc.Bacc("TRN2", target_bir_lowering=False, debug=False,
                   num_devices=cfg.n_cores,
                   dynamic_dma_scratch_size=cfg.dma_scratch,
                   num_swdge_queues=getattr(cfg, "n_queues", 4))

    inT = nc.dram_tensor("inT", [kdim, cfg.rows_core], F32, kind="ExternalInput")
    w = nc.dram_tensor("w", [kdim, feat], F32, kind="ExternalInput")
    biasb = nc.dram_tensor("biasb", [128, feat], F32, kind="ExternalInput")
    iota = nc.dram_tensor("iota", [128, tr], F32, kind="ExternalInput")
    gidx = nc.dram_tensor("gidx", [nt, nb, 128, S16], mybir.dt.int16,
                          kind="ExternalInput")
    meta = nc.dram_tensor("meta", [nt, 128, 2 * NCH], F32, kind="ExternalInput")
    y = nc.dram_tensor("y", [cfg.rows_core, feat], F32, kind="ExternalOutput")

    with tile.TileContext(nc) as tc:
        with tc.tile_pool(name="const", bufs=1) as cp, \
             tc.tile_pool(name="xin", bufs=3) as xp, \
             tc.tile_pool(name="gpool", bufs=2) as gp, \
             tc.tile_pool(name="ipool", bufs=3) as ip, \
             tc.tile_pool(name="mpool", bufs=3) as mp, \
             tc.tile_pool(name="spool", bufs=4) as sp, \
             tc.tile_pool(name="opool", bufs=3) as op, \
             tc.tile_pool(name="psum", bufs=2, space="PSUM") as pp, \
             tc.tile_pool(name="dram", bufs=1, space="DRAM") as dp:

            w_sb = cp.tile([128, kdim], F32, name="w_sb")
            nc.sync.dma_start(
                out=w_sb[:].rearrange("p (kc f) -> p kc f", kc=kc_n),
                in_=w.ap().rearrange("(kc p) f -> p kc f", p=128))
            bias_sb = cp.tile([128, feat], F32, name="bias_sb")
            nc.sync.dma_start(out=bias_sb[:], in_=biasb.ap())
            iota_sb = cp.tile([128, tr], F32, name="iota_sb")
            nc.sync.dma_start(out=iota_sb[:], in_=iota.ap())

            h_slice = []
            h_full = []
            for it in range(cfg.iters):
                if cfg.use_collective:
                    hs = dp.tile([cfg.rows_core, feat], F32,
                                 name=f"h_slice{it}")
                    hf = dp.tile([cfg.n_nodes, feat], F32,
                                 name=f"h_full{it}", addr_space="Shared")
                else:
                    hs = dp.tile([cfg.rows_core, feat], F32,
                                 name=f"h_slice{it}")
                    hf = hs
                h_slice.append(hs)
                h_full.append(hf)

            inT_v = inT.ap().rearrange("(kc p) r -> p kc r", p=128)

            def spmm_out(t, psum, it):
                """PSUM -> SBUF (+bias on last iter) -> DRAM."""
                o = op.tile([tr, feat], F32, name="o")
                if it == cfg.iters:
                    nc.vector.tensor_tensor(out=o[:], in0=psum[:],
                                            in1=bias_sb[:tr, :],
                                            op=mybir.AluOpType.add)
                    nc.sync.dma_start(out=y.ap()[t * tr:(t + 1) * tr, :],
                                      in_=o[:])
                else:
                    nc.vector.tensor_copy(o[:], psum[:])
                    nc.sync.dma_start(
                        out=h_slice[it][t * tr:(t + 1) * tr, :], in_=o[:])

            # ---- phase 1: support = input @ weight (row tiles) ----
            for t in range(nt):
                xin = xp.tile([128, kc_n * tr], F32, name="xin")
                nc.sync.dma_start(
                    out=xin[:].rearrange("p (kc r) -> p kc r", kc=kc_n),
                    in_=inT_v[:, :, t * tr:(t + 1) * tr])
                psum = pp.tile([tr, feat], F32, name="psum1")
                for kc in range(kc_n):
                    nc.tensor.matmul(
                        out=psum[:],
                        lhsT=xin[:, kc * tr:(kc + 1) * tr],
                        rhs=w_sb[:, kc * feat:(kc + 1) * feat],
                        start=(kc == 0), stop=(kc == kc_n - 1))
                spmm_out(t, psum, 0)

            # ---- SpMM iterations ----
            for it in range(1, cfg.iters + 1):
                if cfg.use_collective:
                    nc.gpsimd.collective_compute(
                        "AllGather", mybir.AluOpType.bypass,
                        replica_groups=[list(range(cfg.n_cores))],
                        ins=[h_slice[it - 1][:]],
                        outs=[h_full[it - 1][:]])
                src = h_full[it - 1]
                for t in range(nt):
                    idx_sb = ip.tile([128, nb * S16], mybir.dt.int16,
                                     name="idx_sb")
                    nc.sync.dma_start(
                        out=idx_sb[:].rearrange("p (b s) -> p b s", b=nb),
                        in_=gidx.ap()[t].rearrange("b p s -> p b s"))
                    meta_sb = mp.tile([128, 2 * NCH], F32, name="meta_sb")
                    nc.sync.dma_start(out=meta_sb[:], in_=meta.ap()[t])

                    X = gp.tile([128, NCH * feat], F32, name="X")
                    GC = getattr(cfg, "max_gather_chunks", 5)
                    nq = getattr(cfg, "n_queues", 4)
                    gi = 0
                    for b in range(nb):
                        for g0 in range(0, CB, GC):
                            gch = min(GC, CB - g0)
                            c0 = b * CB + g0
                            nc.gpsimd.dma_gather(
                                out_ap=X[:, c0 * feat:(c0 + gch) * feat]
                                    .rearrange("p (c e) -> p c e", e=feat),
                                in_ap=src[b * cfg.bank_rows:
                                          (b + 1) * cfg.bank_rows, :],
                                idxs_ap=idx_sb[:, b * S16 + g0 * 8:
                                               b * S16 + (g0 + gch) * 8],
                                num_idxs=gch * 128, num_idxs_reg=gch * 128,
                                elem_size=feat, single_packet=False,
                                queue_num=gi % nq)
                            gi += 1

                    psum = pp.tile([tr, feat], F32, name="psum2")
                    for ch in range(NCH):
                        S = sp.tile([128, tr], F32, name="S")
                        nc.vector.tensor_scalar(
                            S[:], iota_sb[:],
                            meta_sb[:, ch:ch + 1],
                            meta_sb[:, NCH + ch:NCH + ch + 1],
                            op0=mybir.AluOpType.is_equal,
                            op1=mybir.AluOpType.mult)
                        nc.tensor.matmul(
                            out=psum[:], lhsT=S[:],
                            rhs=X[:, ch * feat:(ch + 1) * feat],
                            start=(ch == 0), stop=(ch == NCH - 1))
                    spmm_out(t, psum, it if it < cfg.iters else cfg.iters)

    nc.compile()
    return nc


_CACHE = {}
_LAST_RESULT = None


def _get_program(cfg, CB):
    key = (cfg.n_cores, cfg.n_nodes, cfg.tile_rows, cfg.n_banks, cfg.iters, CB)
    if key not in _CACHE:
        _CACHE[key] = _build(cfg, CB)
    return _CACHE[key]


def kernel(input, weight, bias, edge_val, edge_row, edge_col):
    cfg = DEFAULT_CFG
    input = np.asarray(input, dtype=np.float32)
    weight = np.asarray(weight, dtype=np.float32)
    bias = np.asarray(bias, dtype=np.float32)
    edge_val = np.asarray(edge_val, dtype=np.float32)
    edge_row = np.asarray(edge_row, dtype=np.int32)
    edge_col = np.asarray(edge_col, dtype=np.int32)

    in_maps, CB = _preprocess(cfg, input, weight, bias,
                              edge_val, edge_row, edge_col)
    nc = _get_program(cfg, CB)
    res = bass_utils.run_bass_kernel_spmd(
        nc, in_maps, core_ids=list(range(cfg.n_cores)))
    global _LAST_RESULT
    _LAST_RESULT = res
    out = np.concatenate([res.results[m]["y"] for m in range(cfg.n_cores)],
                         axis=0)
    return out
